# revision 9
# baseline (speedup 1.0000x reference)
"""Trainium2 Bass kernel for nn_ALayer_DR1_wh_light_v1 (dense_cnn).

Data-parallel over batch: 16 samples per NeuronCore, no collectives.

Per core, the reconstruct matmul out[o,l] = sum_k wflat[o,k]*y[k]*uf[k,l]*A[l]
is computed as a 3x3 convolution in implicit-GEMM form: the contraction is
reordered to (kh*3+kw major, channel minor) so each 128-row K-tile is one
shift of 128 channels. The rhs tiles are shifted windows of a zero-padded,
bf16 copy of x, scaled by the SE attention y (vector engine broadcast
multiply); spatial attention A is folded into the PSUM->SBUF spill.
N=512 covers 8 samples; the 8 PSUM banks hold the 8 output-channel tiles of
one sample group, accumulated over all 72 K-tiles. Weights stream from HBM
(bf16, host-converted) twice (once per group).
"""
import numpy as np
import ml_dtypes

import concourse.bass as bass
import concourse.mybir as mybir
import concourse.tile as tile
from concourse.bass_utils import run_bass_kernel_spmd
from concourse.vector_clock import ScopedClock

F32 = mybir.dt.float32
BF16 = mybir.dt.bfloat16
AX = mybir.AxisListType.X
AF = mybir.ActivationFunctionType

B, C, H, W = 128, 1024, 8, 8
L = H * W
NCORES = 8
BS = B // NCORES      # 16 samples per core
G = 2                 # sample groups
GB = BS // G          # 8 samples per group
NG = GB * L           # 512 = matmul N per group
NKT = 72              # K tiles (j*8 + ct)
NCT = 8               # channel tiles
NOT = 8               # output channel tiles

TRACE = False
TRACE_DIR = None
LAST_PROFILE = {}

# ---------------------------------------------------------------------------
# Workaround: the staged walrus rejects instructions with >1 sync-wait
# command. Hoist excess waits onto same-engine NOPs inserted before the
# instruction (engine queues issue in order, so semantics are unchanged).
_MAXW = 1
_ctr = [0]


def _split_excess_waits(nc):
    for f in nc.m.functions:
        for blk in f.blocks:
            insts = blk.instructions
            new = []
            changed = False
            for inst in insts:
                si = inst.sync_info
                waits = list(si.on_wait) if si and si.on_wait else []
                if len(waits) > _MAXW:
                    excess = waits[:-_MAXW]
                    si.on_wait = waits[-_MAXW:]
                    while excess:
                        chunk = excess[:_MAXW]
                        excess = excess[_MAXW:]
                        _ctr[0] += 1
                        new.append(mybir.InstNoOp(
                            name=f"I-wsplit-{_ctr[0]}",
                            engine=inst.engine,
                            sync_info=mybir.SyncInfo(on_wait=chunk, on_update=[]),
                        ))
                    changed = True
                new.append(inst)
            if changed:
                insts.clear()
                insts.extend(new)


class _TileContext(tile.TileContext):
    def _drain_and_barrier(self, tick_clock, wait_clock):
        drain_inst = self.nc.sync.drain()
        wait_clock.add_sem_waits(
            drain_inst.ins, ScopedClock({None: tick_clock.global_clock})
        )
        self.nc.all_engine_barrier()
        assert self.sems is not None
        popped = self.nc._tile_sem_poison_stack.pop()
        assert popped is self._sem_poison
        self.nc.clear_and_free_semaphores(list(self.sems.allocated().values()))
        self.nc.all_engine_barrier()


# ---------------------------------------------------------------------------

def _build():
    nc = bass.Bass()
    xs = nc.declare_dram_parameter("xs", [C, BS * L], F32, isOutput=False)
    wtt = nc.declare_dram_parameter("wtt", [NKT, 128, 1024], BF16, isOutput=False)
    w1re = nc.declare_dram_parameter("w1re", [8, C, 512], BF16, isOutput=False)
    f1t = nc.declare_dram_parameter("f1t", [C, 64], BF16, isOutput=False)
    f2pt = nc.declare_dram_parameter("f2pt", [64, 9216], BF16, isOutput=False)
    w2t = nc.declare_dram_parameter("w2t", [512, 8], BF16, isOutput=False)
    out = nc.declare_dram_parameter("out", [NOT, 128, BS * L], F32, isOutput=True)

    with _TileContext(nc) as tc:
        with (
            tc.tile_pool(name="px", bufs=1) as px,
            tc.tile_pool(name="pxpad", bufs=1) as pxpad,
            tc.tile_pool(name="pw", bufs=24) as pw,
            tc.tile_pool(name="pw1", bufs=3) as pw1,
            tc.tile_pool(name="pmod", bufs=6) as pmod,
            tc.tile_pool(name="pout", bufs=3) as pout,
            tc.tile_pool(name="pfix", bufs=1) as pfix,
            tc.tile_pool(name="ps", bufs=1, space="PSUM") as ps,
        ):
            # ---- persistent small tensors
            ybar = pfix.tile([128, NCT * BS], BF16)        # (ct, b)
            xwh = pfix.tile([128, 2 * NCT * 128], BF16)    # xw | xh, (ct, b, v)
            ys = pfix.tile([128, NKT * BS], BF16)          # (kt, b)
            f1sb = pfix.tile([128, NCT * 64], BF16)
            f2sb = pfix.tile([64, 9216], BF16)
            w2sb = pfix.tile([128, 4 * 8], BF16)
            t1 = pfix.tile([64, BS], BF16)
            z1 = pfix.tile([128, 4 * 2 * BS], BF16)        # (mt, path*b)
            ahw = pfix.tile([8, 2 * BS], F32)
            ahrow = pfix.tile([1, 128], F32)
            awrow = pfix.tile([1, 128], F32)
            awx = pfix.tile([1, BS * L], F32)
            arow = pfix.tile([1, BS * L], F32)
            ones = pfix.tile([1, 128], F32)
            afull = pfix.tile([128, BS * L], F32)
            xpad = [pxpad.tile([128, BS * 100], BF16, tag=f"xp{ct}", name=f"xp{ct}")
                    for ct in range(NCT)]

            nc.gpsimd.memset(ones[:], 1.0)

            # ---- weight-ish DMAs
            # f1sb[p, ct*64+c2] = f1t[ct*128+p, c2]
            nc.sync.dma_start(
                out=f1sb[:].rearrange("p (ct c2) -> p ct c2", ct=NCT),
                in_=f1t.rearrange("(ct p) c2 -> p ct c2", ct=NCT))
            nc.sync.dma_start(out=f2sb[:], in_=f2pt[:])
            nc.sync.dma_start(
                out=w2sb[:].rearrange("p (mt c2) -> p mt c2", mt=4),
                in_=w2t.rearrange("(mt p) c2 -> p mt c2", mt=4))

            kt_order = ([4 * NCT + c for c in range(NCT)]
                        + [j * NCT + c for j in range(9) if j != 4
                           for c in range(NCT)])
            # early weight streams: first w1 chunks + 24 conv-weight slices so
            # the HWDGE queues carry them alongside the attention inputs
            w1sbs = []
            for v in range(3):
                w1sb = pw1.tile([128, NCT * 512], BF16, tag="w1sb",
                                name=f"w1sb{v}")
                nc.sync.dma_start(
                    out=w1sb[:].rearrange("p (ct m) -> p ct m", ct=NCT),
                    in_=w1re[v].rearrange("(ct p) m -> p ct m", ct=NCT))
                w1sbs.append(w1sb)
            NPRE = 24
            wsl_pre = []
            for i in range(NPRE):
                wsl = pw.tile([128, 1024], BF16, tag="wsl", name=f"wslp{i}")
                nc.sync.dma_start(out=wsl[:], in_=wtt[kt_order[i]])
                wsl_pre.append(wsl)

            # ---- per-channel-tile input processing
            # (ybar + xpad first: they gate the y path and the conv; the
            # xw/xh reductions for the spatial path can lag)
            xfs = []
            for ct in range(NCT):
                xf = px.tile([128, BS * L], F32, tag=f"xf{ct}", name=f"xf{ct}")
                xfs.append(xf)
                nc.sync.dma_start(out=xf[:], in_=xs[ct * 128:(ct + 1) * 128, :])
                with nc.allow_low_precision(reason="bf16 activations"):
                    nc.vector.reduce_sum(
                        ybar[:, ct * BS:(ct + 1) * BS],
                        xf[:].rearrange("p (b l) -> p b l", b=BS), axis=AX)
                nc.gpsimd.memset(xpad[ct][:], 0.0)
                nc.vector.tensor_copy(
                    xpad[ct][:].rearrange(
                        "p (b hh ww) -> p b hh ww", b=BS, hh=10, ww=10)
                    [:, :, 1:9, 1:9],
                    xf[:].rearrange("p (b h w) -> p b h w", b=BS, h=H, w=W))
            for ct in range(NCT):
                xf = xfs[ct]
                with nc.allow_low_precision(reason="bf16 activations"):
                    nc.vector.reduce_sum(
                        xwh[:, ct * 128:(ct + 1) * 128]
                        .rearrange("p (b v) -> p b v", b=BS),
                        xf[:].rearrange("p (b h v) -> p b v h", b=BS, h=H, v=W),
                        axis=AX)
                    nc.vector.reduce_sum(
                        xwh[:, 1024 + ct * 128:1024 + (ct + 1) * 128]
                        .rearrange("p (b v) -> p b v", b=BS),
                        xf[:].rearrange("p (b v w) -> p b v w", b=BS, v=H, w=W),
                        axis=AX)

            # ---- SE channel-kernel attention y
            psy = ps.tile([64, BS], F32, tag="cv0")
            for ct in range(NCT):
                nc.tensor.matmul(
                    psy[:], f1sb[:, ct * 64:(ct + 1) * 64],
                    ybar[:, ct * BS:(ct + 1) * BS],
                    start=(ct == 0), stop=(ct == NCT - 1))
            nc.scalar.activation(t1[:], psy[:], AF.Relu)
            for mq in range(NKT // 4):
                psf = ps.tile([128, 4 * BS], F32, tag=f"cv{1 + (mq % 3)}",
                              name=f"psf{mq}")
                for sub in range(4):
                    mt = mq * 4 + sub
                    nc.tensor.matmul(
                        psf[:, sub * BS:(sub + 1) * BS],
                        f2sb[:, mt * 128:(mt + 1) * 128], t1[:],
                        start=True, stop=True)
                nc.scalar.activation(
                    ys[:, mq * 4 * BS:(mq + 1) * 4 * BS], psf[:], AF.Sigmoid)

            # ---- spatial attention Aw/Ah
            psz = [ps.tile([128, 2 * BS], F32, tag=f"cv{4 + mt}", name=f"psz{mt}")
                   for mt in range(4)]
            for v in range(8):
                if v < 3:
                    w1sb = w1sbs[v]
                else:
                    w1sb = pw1.tile([128, NCT * 512], BF16, tag="w1sb",
                                    name=f"w1sb{v}")
                    nc.sync.dma_start(
                        out=w1sb[:].rearrange("p (ct m) -> p ct m", ct=NCT),
                        in_=w1re[v].rearrange("(ct p) m -> p ct m", ct=NCT))
                for ct in range(NCT):
                    rhs = xwh[:].rearrange(
                        "p (path ct b v) -> p ct v path b",
                        path=2, ct=NCT, b=BS, v=8)[:, ct, v]
                    for mt in range(4):
                        nc.tensor.matmul(
                            psz[mt][:],
                            w1sb[:, ct * 512 + mt * 128:ct * 512 + (mt + 1) * 128],
                            rhs,
                            start=(v == 0 and ct == 0),
                            stop=(v == 7 and ct == NCT - 1))
            for mt in range(4):
                nc.scalar.activation(
                    z1[:, mt * 2 * BS:(mt + 1) * 2 * BS], psz[mt][:], AF.Relu)
            psa = ps.tile([8, 2 * BS], F32, tag="cv0")
            for mt in range(4):
                nc.tensor.matmul(
                    psa[:], w2sb[:, mt * 8:(mt + 1) * 8],
                    z1[:, mt * 2 * BS:(mt + 1) * 2 * BS],
                    start=(mt == 0), stop=(mt == 3))
            nc.scalar.activation(ahw[:], psa[:], AF.Sigmoid)
            # rows stored (v, b): awrow[w*16+b] = Aw[b,w]; ahrow[h*16+b]
            nc.sync.dma_start(out=awrow[:], in_=ahw[:, 0:BS])
            nc.sync.dma_start(out=ahrow[:], in_=ahw[:, BS:2 * BS])
            # awx[(b,h,w)] = aw[b,w] ; write in (b,w,h) enumeration order
            nc.vector.tensor_copy(
                awx[:].rearrange("p (b h w) -> p b w h", b=BS, h=H, w=W),
                awrow[:].rearrange("p (w b) -> p b w", w=W)
                .broadcast_to([1, BS, W, H]))
            nc.vector.tensor_mul(
                arow[:].rearrange("p (b h w) -> p b h w", b=BS, h=H, w=W),
                ahrow[:].rearrange("p (h b) -> p b h", h=H)
                .broadcast_to([1, BS, H, W]),
                awx[:].rearrange("p (b h w) -> p b h w", b=BS, h=H, w=W))
            # broadcast across partitions via ones-matmul
            for half in range(2):
                psb = ps.tile([128, NG], F32, tag=f"cv{1 + half}")
                nc.tensor.matmul(
                    psb[:], ones[:], arow[:, half * NG:(half + 1) * NG],
                    start=True, stop=True)
                nc.vector.tensor_copy(
                    afull[:, half * NG:(half + 1) * NG], psb[:])

            # ---- main conv
            for g in range(G):
                pscv = [ps.tile([128, NG], F32, tag=f"cv{ot}", name=f"pscv{g}_{ot}")
                        for ot in range(NOT)]
                for idx, kt in enumerate(kt_order):
                    j, ct = kt // NCT, kt % NCT
                    dh, dw = j // 3, j % 3
                    h0 = 1 if dh == 0 else 0
                    nh = 8 - (1 if dh != 1 else 0)
                    w0 = 1 if dw == 0 else 0
                    nw = 8 - (1 if dw != 1 else 0)
                    if g == 0 and idx < NPRE:
                        wsl = wsl_pre[idx]
                    else:
                        wsl = pw.tile([128, 1024], BF16, tag="wsl")
                        nc.sync.dma_start(out=wsl[:], in_=wtt[kt])
                    md = pmod.tile([128, NG], BF16, tag="md")
                    nc.vector.tensor_mul(
                        md[:].rearrange("p (b h w) -> p b h w", b=GB, h=H, w=W)
                        [:, :, h0:h0 + nh, w0:w0 + nw],
                        xpad[ct][:].rearrange(
                            "p (b hh ww) -> p b hh ww", b=BS, hh=10, ww=10)
                        [:, g * GB:(g + 1) * GB,
                         dh + h0:dh + h0 + nh, dw + w0:dw + w0 + nw],
                        ys[:, kt * BS + g * GB:kt * BS + (g + 1) * GB]
                        .broadcast_to([128, GB, nh, nw]))
                    for ot in range(NOT):
                        nc.tensor.matmul(
                            pscv[ot][:]
                            .rearrange("p (b h w) -> p b h w", b=GB, h=H, w=W)
                            [:, :, h0:h0 + nh, w0:w0 + nw],
                            wsl[:, ot * 128:(ot + 1) * 128],
                            md[:].rearrange("p (b h w) -> p b h w",
                                            b=GB, h=H, w=W)
                            [:, :, h0:h0 + nh, w0:w0 + nw],
                            start=(idx == 0), stop=(idx == NKT - 1))
                for ot in range(NOT):
                    ob = pout.tile([128, NG], F32, tag="ob")
                    nc.vector.tensor_mul(
                        ob[:], pscv[ot][:],
                        afull[:, g * NG:(g + 1) * NG])
                    nc.sync.dma_start(
                        out=out[ot, :, g * NG:(g + 1) * NG], in_=ob[:])

    _split_excess_waits(nc)
    return nc


_NC_CACHE = []


def kernel(x, weight, w1, w2, f1, f2):
    global LAST_PROFILE
    bf = ml_dtypes.bfloat16

    # host-side weight reorders (shared by all cores)
    wtt = np.ascontiguousarray(
        weight.reshape(C, C, 9).transpose(2, 1, 0).reshape(NKT, 128, 8, 128)
        .reshape(NKT, 128, 1024)).astype(bf)
    w1re = np.ascontiguousarray(
        (w1.reshape(512, C, 8).transpose(2, 1, 0) / 8.0)).astype(bf)
    f1t = np.ascontiguousarray(f1.T / 64.0).astype(bf)
    f2pt = np.ascontiguousarray(
        f2.reshape(C, 9, 64).transpose(1, 0, 2).reshape(9216, 64).T).astype(bf)
    w2t = np.ascontiguousarray(w2.T).astype(bf)

    in_maps = []
    for i in range(NCORES):
        xsh = x[i * BS:(i + 1) * BS]                      # [16, C, H, W]
        xs = np.ascontiguousarray(
            xsh.transpose(1, 0, 2, 3).reshape(C, BS * L)).astype(np.float32)
        in_maps.append(dict(xs=xs, wtt=wtt, w1re=w1re, f1t=f1t,
                            f2pt=f2pt, w2t=w2t))

    if not _NC_CACHE:
        _NC_CACHE.append(_build())
    nc = _NC_CACHE[0]

    kw = {}
    if TRACE:
        kw = dict(trace=True, tmpdir=TRACE_DIR)
    r = run_bass_kernel_spmd(nc, in_maps, core_ids=list(range(NCORES)), **kw)
    if TRACE:
        LAST_PROFILE = dict(
            exec_time_ns=r.exec_time_ns,
            mean_exec_time_ns=r.mean_exec_time_ns,
            profile_json=r.profile_json,
            trace=(r.instructions_and_trace[1]
                   if r.instructions_and_trace else None),
        )

    out = np.empty((B, C, H, W), np.float32)
    for i in range(NCORES):
        res = r.results[i]["out"]                         # [8, 128, BS*L]
        out[i * BS:(i + 1) * BS] = (
            res.reshape(NOT, 128, BS, L).transpose(2, 0, 1, 3)
            .reshape(BS, C, H, W))
    return out


# revision 10
# speedup vs baseline: 1.0071x; 1.0071x over previous
"""Trainium2 Bass kernel for nn_ALayer_DR1_wh_light_v1 (dense_cnn).

Data-parallel over batch: 16 samples per NeuronCore, no collectives.

Per core, the reconstruct matmul out[o,l] = sum_k wflat[o,k]*y[k]*uf[k,l]*A[l]
is computed as a 3x3 convolution in implicit-GEMM form: the contraction is
reordered to (kh*3+kw major, channel minor) so each 128-row K-tile is one
shift of 128 channels. The rhs tiles are shifted windows of a zero-padded,
bf16 copy of x, scaled by the SE attention y (vector engine broadcast
multiply); spatial attention A is folded into the PSUM->SBUF spill.
N=512 covers 8 samples; the 8 PSUM banks hold the 8 output-channel tiles of
one sample group, accumulated over all 72 K-tiles. Weights stream from HBM
(bf16, host-converted) twice (once per group).
"""
import numpy as np
import ml_dtypes

import concourse.bass as bass
import concourse.mybir as mybir
import concourse.tile as tile
from concourse.bass_utils import run_bass_kernel_spmd
from concourse.vector_clock import ScopedClock

F32 = mybir.dt.float32
BF16 = mybir.dt.bfloat16
AX = mybir.AxisListType.X
AF = mybir.ActivationFunctionType

B, C, H, W = 128, 1024, 8, 8
L = H * W
NCORES = 8
BS = B // NCORES      # 16 samples per core
G = 2                 # sample groups
GB = BS // G          # 8 samples per group
NG = GB * L           # 512 = matmul N per group
NKT = 72              # K tiles (j*8 + ct)
NCT = 8               # channel tiles
NOT = 8               # output channel tiles

TRACE = False
TRACE_DIR = None
LAST_PROFILE = {}

# ---------------------------------------------------------------------------
# Workaround: the staged walrus rejects instructions with >1 sync-wait
# command. Hoist excess waits onto same-engine NOPs inserted before the
# instruction (engine queues issue in order, so semantics are unchanged).
_MAXW = 1
_ctr = [0]


def _split_excess_waits(nc):
    for f in nc.m.functions:
        for blk in f.blocks:
            insts = blk.instructions
            new = []
            changed = False
            for inst in insts:
                si = inst.sync_info
                waits = list(si.on_wait) if si and si.on_wait else []
                if len(waits) > _MAXW:
                    excess = waits[:-_MAXW]
                    si.on_wait = waits[-_MAXW:]
                    while excess:
                        chunk = excess[:_MAXW]
                        excess = excess[_MAXW:]
                        _ctr[0] += 1
                        new.append(mybir.InstNoOp(
                            name=f"I-wsplit-{_ctr[0]}",
                            engine=inst.engine,
                            sync_info=mybir.SyncInfo(on_wait=chunk, on_update=[]),
                        ))
                    changed = True
                new.append(inst)
            if changed:
                insts.clear()
                insts.extend(new)


class _TileContext(tile.TileContext):
    def _drain_and_barrier(self, tick_clock, wait_clock):
        drain_inst = self.nc.sync.drain()
        wait_clock.add_sem_waits(
            drain_inst.ins, ScopedClock({None: tick_clock.global_clock})
        )
        self.nc.all_engine_barrier()
        assert self.sems is not None
        popped = self.nc._tile_sem_poison_stack.pop()
        assert popped is self._sem_poison
        self.nc.clear_and_free_semaphores(list(self.sems.allocated().values()))
        self.nc.all_engine_barrier()


# ---------------------------------------------------------------------------

def _build():
    nc = bass.Bass()
    xsp = nc.declare_dram_parameter("xsp", [C, BS * 100], BF16, isOutput=False)
    wtt = nc.declare_dram_parameter("wtt", [NKT, 128, 1024], BF16, isOutput=False)
    w1re = nc.declare_dram_parameter("w1re", [8, C, 512], BF16, isOutput=False)
    f1t = nc.declare_dram_parameter("f1t", [C, 64], BF16, isOutput=False)
    f2pt = nc.declare_dram_parameter("f2pt", [64, 9216], BF16, isOutput=False)
    w2t = nc.declare_dram_parameter("w2t", [512, 8], BF16, isOutput=False)
    out = nc.declare_dram_parameter("out", [NOT, 128, BS * L], F32, isOutput=True)

    with _TileContext(nc) as tc:
        with (
            tc.tile_pool(name="pxpad", bufs=1) as pxpad,
            tc.tile_pool(name="pw", bufs=24) as pw,
            tc.tile_pool(name="pw1", bufs=3) as pw1,
            tc.tile_pool(name="pmod", bufs=6) as pmod,
            tc.tile_pool(name="pout", bufs=3) as pout,
            tc.tile_pool(name="pfix", bufs=1) as pfix,
            tc.tile_pool(name="ps", bufs=1, space="PSUM") as ps,
        ):
            # ---- persistent small tensors
            ybar = pfix.tile([128, NCT * BS], BF16)        # (ct, b)
            xwh = pfix.tile([128, 2 * NCT * 128], BF16)    # xw | xh, (ct, b, v)
            ys = pfix.tile([128, NKT * BS], BF16)          # (kt, b)
            f1sb = pfix.tile([128, NCT * 64], BF16)
            f2sb = pfix.tile([64, 9216], BF16)
            w2sb = pfix.tile([128, 4 * 8], BF16)
            t1 = pfix.tile([64, BS], BF16)
            z1 = pfix.tile([128, 4 * 2 * BS], BF16)        # (mt, path*b)
            ahw = pfix.tile([8, 2 * BS], F32)
            ahrow = pfix.tile([1, 128], F32)
            awrow = pfix.tile([1, 128], F32)
            awx = pfix.tile([1, BS * L], F32)
            arow = pfix.tile([1, BS * L], F32)
            ones = pfix.tile([1, 128], F32)
            afull = pfix.tile([128, BS * L], F32)
            xpad = [pxpad.tile([128, BS * 100], BF16, tag=f"xp{ct}", name=f"xp{ct}")
                    for ct in range(NCT)]

            nc.gpsimd.memset(ones[:], 1.0)

            # ---- weight-ish DMAs
            # f1sb[p, ct*64+c2] = f1t[ct*128+p, c2]
            nc.sync.dma_start(
                out=f1sb[:].rearrange("p (ct c2) -> p ct c2", ct=NCT),
                in_=f1t.rearrange("(ct p) c2 -> p ct c2", ct=NCT))
            nc.sync.dma_start(out=f2sb[:], in_=f2pt[:])
            nc.sync.dma_start(
                out=w2sb[:].rearrange("p (mt c2) -> p mt c2", mt=4),
                in_=w2t.rearrange("(mt p) c2 -> p mt c2", mt=4))

            kt_order = ([4 * NCT + c for c in range(NCT)]
                        + [j * NCT + c for j in range(9) if j != 4
                           for c in range(NCT)])
            # early weight streams: first w1 chunks + 24 conv-weight slices so
            # the HWDGE queues carry them alongside the attention inputs
            w1sbs = []
            for v in range(3):
                w1sb = pw1.tile([128, NCT * 512], BF16, tag="w1sb",
                                name=f"w1sb{v}")
                nc.sync.dma_start(
                    out=w1sb[:].rearrange("p (ct m) -> p ct m", ct=NCT),
                    in_=w1re[v].rearrange("(ct p) m -> p ct m", ct=NCT))
                w1sbs.append(w1sb)
            NPRE = 24
            wsl_pre = []
            for i in range(NPRE):
                wsl = pw.tile([128, 1024], BF16, tag="wsl", name=f"wslp{i}")
                nc.sync.dma_start(out=wsl[:], in_=wtt[kt_order[i]])
                wsl_pre.append(wsl)

            # ---- per-channel-tile input processing (x arrives pre-padded
            # bf16 from the host; xh -> ybar chain gates the y path)
            for ct in range(NCT):
                nc.sync.dma_start(
                    out=xpad[ct][:], in_=xsp[ct * 128:(ct + 1) * 128, :])
            with nc.allow_low_precision(reason="bf16 activations"):
                for ct in range(NCT):
                    xhsl = xwh[:, 1024 + ct * 128:1024 + (ct + 1) * 128]
                    nc.vector.reduce_sum(
                        xhsl.rearrange("p (b v) -> p b v", b=BS),
                        xpad[ct][:].rearrange(
                            "p (b hh ww) -> p b hh ww", b=BS, hh=10, ww=10)
                        [:, :, 1:9, 1:9],
                        axis=AX)
                    nc.vector.reduce_sum(
                        ybar[:, ct * BS:(ct + 1) * BS],
                        xhsl.rearrange("p (b v) -> p b v", b=BS), axis=AX)
                for ct in range(NCT):
                    nc.vector.reduce_sum(
                        xwh[:, ct * 128:(ct + 1) * 128]
                        .rearrange("p (b v) -> p b v", b=BS),
                        xpad[ct][:].rearrange(
                            "p (b hh ww) -> p b ww hh", b=BS, hh=10, ww=10)
                        [:, :, 1:9, 1:9],
                        axis=AX)

            # ---- SE channel-kernel attention y
            psy = ps.tile([64, BS], F32, tag="cv0")
            for ct in range(NCT):
                nc.tensor.matmul(
                    psy[:], f1sb[:, ct * 64:(ct + 1) * 64],
                    ybar[:, ct * BS:(ct + 1) * BS],
                    start=(ct == 0), stop=(ct == NCT - 1))
            nc.scalar.activation(t1[:], psy[:], AF.Relu)
            for mq in range(NKT // 4):
                psf = ps.tile([128, 4 * BS], F32, tag=f"cv{1 + (mq % 3)}",
                              name=f"psf{mq}")
                for sub in range(4):
                    mt = mq * 4 + sub
                    nc.tensor.matmul(
                        psf[:, sub * BS:(sub + 1) * BS],
                        f2sb[:, mt * 128:(mt + 1) * 128], t1[:],
                        start=True, stop=True)
                nc.scalar.activation(
                    ys[:, mq * 4 * BS:(mq + 1) * 4 * BS], psf[:], AF.Sigmoid)

            # ---- spatial attention Aw/Ah
            psz = [ps.tile([128, 2 * BS], F32, tag=f"cv{4 + mt}", name=f"psz{mt}")
                   for mt in range(4)]
            for v in range(8):
                if v < 3:
                    w1sb = w1sbs[v]
                else:
                    w1sb = pw1.tile([128, NCT * 512], BF16, tag="w1sb",
                                    name=f"w1sb{v}")
                    nc.sync.dma_start(
                        out=w1sb[:].rearrange("p (ct m) -> p ct m", ct=NCT),
                        in_=w1re[v].rearrange("(ct p) m -> p ct m", ct=NCT))
                for ct in range(NCT):
                    rhs = xwh[:].rearrange(
                        "p (path ct b v) -> p ct v path b",
                        path=2, ct=NCT, b=BS, v=8)[:, ct, v]
                    for mt in range(4):
                        nc.tensor.matmul(
                            psz[mt][:],
                            w1sb[:, ct * 512 + mt * 128:ct * 512 + (mt + 1) * 128],
                            rhs,
                            start=(v == 0 and ct == 0),
                            stop=(v == 7 and ct == NCT - 1))
            for mt in range(4):
                nc.scalar.activation(
                    z1[:, mt * 2 * BS:(mt + 1) * 2 * BS], psz[mt][:], AF.Relu)
            psa = ps.tile([8, 2 * BS], F32, tag="cv0")
            for mt in range(4):
                nc.tensor.matmul(
                    psa[:], w2sb[:, mt * 8:(mt + 1) * 8],
                    z1[:, mt * 2 * BS:(mt + 1) * 2 * BS],
                    start=(mt == 0), stop=(mt == 3))
            nc.scalar.activation(ahw[:], psa[:], AF.Sigmoid)
            # rows stored (v, b): awrow[w*16+b] = Aw[b,w]; ahrow[h*16+b]
            nc.sync.dma_start(out=awrow[:], in_=ahw[:, 0:BS])
            nc.sync.dma_start(out=ahrow[:], in_=ahw[:, BS:2 * BS])
            # awx[(b,h,w)] = aw[b,w] ; write in (b,w,h) enumeration order
            nc.vector.tensor_copy(
                awx[:].rearrange("p (b h w) -> p b w h", b=BS, h=H, w=W),
                awrow[:].rearrange("p (w b) -> p b w", w=W)
                .broadcast_to([1, BS, W, H]))
            nc.vector.tensor_mul(
                arow[:].rearrange("p (b h w) -> p b h w", b=BS, h=H, w=W),
                ahrow[:].rearrange("p (h b) -> p b h", h=H)
                .broadcast_to([1, BS, H, W]),
                awx[:].rearrange("p (b h w) -> p b h w", b=BS, h=H, w=W))
            # broadcast across partitions via ones-matmul
            for half in range(2):
                psb = ps.tile([128, NG], F32, tag=f"cv{1 + half}")
                nc.tensor.matmul(
                    psb[:], ones[:], arow[:, half * NG:(half + 1) * NG],
                    start=True, stop=True)
                nc.vector.tensor_copy(
                    afull[:, half * NG:(half + 1) * NG], psb[:])

            # ---- main conv
            for g in range(G):
                pscv = [ps.tile([128, NG], F32, tag=f"cv{ot}", name=f"pscv{g}_{ot}")
                        for ot in range(NOT)]
                for idx, kt in enumerate(kt_order):
                    j, ct = kt // NCT, kt % NCT
                    dh, dw = j // 3, j % 3
                    h0 = 1 if dh == 0 else 0
                    nh = 8 - (1 if dh != 1 else 0)
                    w0 = 1 if dw == 0 else 0
                    nw = 8 - (1 if dw != 1 else 0)
                    if g == 0 and idx < NPRE:
                        wsl = wsl_pre[idx]
                    else:
                        wsl = pw.tile([128, 1024], BF16, tag="wsl")
                        nc.sync.dma_start(out=wsl[:], in_=wtt[kt])
                    md = pmod.tile([128, NG], BF16, tag="md")
                    nc.vector.tensor_mul(
                        md[:].rearrange("p (b h w) -> p b h w", b=GB, h=H, w=W)
                        [:, :, h0:h0 + nh, w0:w0 + nw],
                        xpad[ct][:].rearrange(
                            "p (b hh ww) -> p b hh ww", b=BS, hh=10, ww=10)
                        [:, g * GB:(g + 1) * GB,
                         dh + h0:dh + h0 + nh, dw + w0:dw + w0 + nw],
                        ys[:, kt * BS + g * GB:kt * BS + (g + 1) * GB]
                        .broadcast_to([128, GB, nh, nw]))
                    for ot in range(NOT):
                        nc.tensor.matmul(
                            pscv[ot][:]
                            .rearrange("p (b h w) -> p b h w", b=GB, h=H, w=W)
                            [:, :, h0:h0 + nh, w0:w0 + nw],
                            wsl[:, ot * 128:(ot + 1) * 128],
                            md[:].rearrange("p (b h w) -> p b h w",
                                            b=GB, h=H, w=W)
                            [:, :, h0:h0 + nh, w0:w0 + nw],
                            start=(idx == 0), stop=(idx == NKT - 1))
                for ot in range(NOT):
                    ob = pout.tile([128, NG], F32, tag="ob")
                    nc.vector.tensor_mul(
                        ob[:], pscv[ot][:],
                        afull[:, g * NG:(g + 1) * NG])
                    nc.sync.dma_start(
                        out=out[ot, :, g * NG:(g + 1) * NG], in_=ob[:])

    _split_excess_waits(nc)
    return nc


_NC_CACHE = []


def kernel(x, weight, w1, w2, f1, f2):
    global LAST_PROFILE
    bf = ml_dtypes.bfloat16

    # host-side weight reorders (shared by all cores)
    wtt = np.ascontiguousarray(
        weight.reshape(C, C, 9).transpose(2, 1, 0).reshape(NKT, 128, 8, 128)
        .reshape(NKT, 128, 1024)).astype(bf)
    w1re = np.ascontiguousarray(
        (w1.reshape(512, C, 8).transpose(2, 1, 0) / 8.0)).astype(bf)
    f1t = np.ascontiguousarray(f1.T / 64.0).astype(bf)
    f2pt = np.ascontiguousarray(
        f2.reshape(C, 9, 64).transpose(1, 0, 2).reshape(9216, 64).T).astype(bf)
    w2t = np.ascontiguousarray(w2.T).astype(bf)

    in_maps = []
    for i in range(NCORES):
        xsh = x[i * BS:(i + 1) * BS]                      # [16, C, H, W]
        xsp = np.zeros((C, BS, 10, 10), bf)
        xsp[:, :, 1:9, 1:9] = xsh.transpose(1, 0, 2, 3).astype(bf)
        in_maps.append(dict(xsp=xsp.reshape(C, BS * 100), wtt=wtt, w1re=w1re,
                            f1t=f1t, f2pt=f2pt, w2t=w2t))

    if not _NC_CACHE:
        _NC_CACHE.append(_build())
    nc = _NC_CACHE[0]

    kw = {}
    if TRACE:
        kw = dict(trace=True, tmpdir=TRACE_DIR)
    r = run_bass_kernel_spmd(nc, in_maps, core_ids=list(range(NCORES)), **kw)
    if TRACE:
        LAST_PROFILE = dict(
            exec_time_ns=r.exec_time_ns,
            mean_exec_time_ns=r.mean_exec_time_ns,
            profile_json=r.profile_json,
            trace=(r.instructions_and_trace[1]
                   if r.instructions_and_trace else None),
        )

    out = np.empty((B, C, H, W), np.float32)
    for i in range(NCORES):
        res = r.results[i]["out"]                         # [8, 128, BS*L]
        out[i * BS:(i + 1) * BS] = (
            res.reshape(NOT, 128, BS, L).transpose(2, 0, 1, 3)
            .reshape(BS, C, H, W))
    return out
